# revision 6
# baseline (speedup 1.0000x reference)
"""MatchNet kernel for 8 Trainium2 NeuronCores.

Math (per batch b):
    keys   = q[b] @ W + bias
    scores = p[b] @ keys^T
    attn   = softmax(scores, axis=-1)
    out[b] = relu(attn @ q[b])

The Dense bias is dropped: softmax over lq is invariant to the per-lp
constant p@b^T it adds to scores, and keys are not used elsewhere.

Sharding: data-parallel over B=16 across 8 cores (2 batches per core).
W is broadcast. p and q are transposed on the host so every on-chip matmul
has its contraction dim on SBUF partitions.

Precision: the softmax is extremely sharp (scores std ~32), so plain
bf16/fp16 matmuls in the score path are not accurate enough. Each score-path
matmul runs as:
  - 1 fp16 "main" pass: x16 @ y16 at full PE rate (8 k-chunk matmuls), and
  - 1 single-term fp8-e5m2 correction at 2x PE rate via DoubleRow, with the
    pair slots carrying two CONSECUTIVE contraction chunks (4 matmuls):
      MM1 += qr8 @ W8        (q residual vs fp16, e5m2)
      MM2 += pr8 @ k8        (p residual vs fp16, e5m2)
    DoubleRow computes sum_i lhsT[:,i,:].T @ rhs[:,i,:].
The other two correction terms (W residual, keys residual) are dropped:
numpy sim of this exact scheme on the real inputs: rel_err 1.0e-2
(gate 2e-2; HW has measured slightly BETTER than sim on two prior configs).
    MM1: keysT[h, lq] = sum_hk W[hk, h] * qT[hk, lq]
         keysT split on-chip: k16 (fp16) + k8 (e5m2) via DVE copies
    MM2: scores[lp, lq] = sum_h pT[h, lp] * keysT[h, lq]
    softmax over free dim; exp via ACT (bias=-rowmax, accum rowsum),
    exp output stored fp16
    T:   attnT[lq, lp] via PE transpose (fp16)
    MM3: out[lp, h] = sum_lq attnT[lq, lp] * q[lq, h]  single fp16 pass
    relu(out * (1/rowsum)) via ACT with per-partition scale
(DMA xbar transpose for attnT was tried instead of PE transposes and was
~110us SLOWER end-to-end — keep the PE-transpose path.)
"""

import os
from contextlib import ExitStack

import ml_dtypes
import numpy as np

import concourse.bass as bass
import concourse.mybir as mybir
import concourse.tile as tile
from concourse import bacc
from concourse.bass import ts
from concourse.bass_utils import run_bass_kernel_spmd
from concourse.masks import make_identity

B, L, H = 16, 1024, 1024
NCORES = 8
BPC = B // NCORES  # batches per core
P = 128
KO = H // P        # 8 contraction chunks
KD = KO // 2       # 4 DoubleRow chunk-pairs
NT = L // P        # 8 lp tiles per batch
NF = 512           # matmul moving free dim
NCH = L // NF      # 2 free chunks
F32 = mybir.dt.float32
F16 = mybir.dt.float16
E5 = mybir.dt.float8e5
AF = mybir.ActivationFunctionType
AX = mybir.AxisListType
DR = mybir.MatmulPerfMode.DoubleRow


def _build_body(ctx, tc, ins, out):
    nc = tc.nc
    pT16, pT8r, qT16, qT8r, qn16, W16, W8 = ins

    # PE warmup: the first ~15us are DMA-bound (bootstrap + first loads) and
    # the PE would sit idle, entering the kernel HAM-throttled at 1.2 GHz.
    # Zero matmuls during that window cost nothing and flip the clock gate
    # to 2.4 GHz before the real matmuls start.
    with (
        tc.tile_pool(name="warm", bufs=1) as warm_pool,
        tc.tile_pool(name="warmps", bufs=1, space=bass.MemorySpace.PSUM) as wps_pool,
    ):
        wsb = warm_pool.tile([P, P], F16)
        nc.gpsimd.memset(wsb[:], 0.0)
        wps = wps_pool.tile([P, P], F32)
        for _ in range(60):
            nc.tensor.matmul(wps[:], wsb[:], wsb[:], start=True, stop=True)

    const = ctx.enter_context(tc.tile_pool(name="const", bufs=1))
    # W tiles, one per k-chunk (chunk-granular deps let the first matmul
    # start as soon as chunk 0 lands instead of after the full load).
    W16_sb = [const.tile([P, H], F16, name=f"W16_sb_{k}") for k in range(KO)]
    # W8 pair tiles: slot s holds W8 rows [256*kk + 128*s, +128)
    W8_sb = [const.tile([P, 2, H], E5, name=f"W8_sb_{kk}") for kk in range(KD)]
    ident = const.tile([P, P], F16)
    make_identity(nc, ident[:])

    qT_pool = ctx.enter_context(tc.tile_pool(name="qTp", bufs=1))
    q_pool = ctx.enter_context(tc.tile_pool(name="qp", bufs=1))
    keysT_pool = ctx.enter_context(tc.tile_pool(name="keysTp", bufs=1))
    pT_pool = ctx.enter_context(tc.tile_pool(name="pTp", bufs=3))
    attn_pool = ctx.enter_context(tc.tile_pool(name="attnp", bufs=2))
    attnT_pool = ctx.enter_context(tc.tile_pool(name="attnTp", bufs=2))
    osb_pool = ctx.enter_context(tc.tile_pool(name="osbp", bufs=2))
    stat_pool = ctx.enter_context(tc.tile_pool(name="statp", bufs=8))
    ps_big = ctx.enter_context(
        tc.tile_pool(name="psbig", bufs=3, space=bass.MemorySpace.PSUM)
    )
    ps_t = ctx.enter_context(
        tc.tile_pool(name="pst", bufs=2, space=bass.MemorySpace.PSUM)
    )

    W16_re = W16.rearrange("(ko ki) h -> ki ko h", ki=P)
    W8_re = W8.rearrange("(kk two ki) h -> ki kk two h", ki=P, two=2)

    for b in range(BPC):
        # qT tiles first (MM1 needs them); fp16 mains before fp8 residuals so
        # the first fp16 matmul of each k-chunk can start while residuals load.
        qT16_sb = [
            qT_pool.tile([P, L], F16, name=f"qT16_sb_{b}_{k}", tag=f"qT16_sb{k}")
            for k in range(KO)
        ]
        qT8r_sb = [
            qT_pool.tile([P, 2, L], E5, name=f"qT8r_sb_{b}_{kk}", tag=f"qT8r_sb{kk}")
            for kk in range(KD)
        ]
        qT16_re = qT16[b].rearrange("(ko ki) l -> ki ko l", ki=P)
        qT8r_re = qT8r[b].rearrange("(kk two ki) l -> ki kk two l", ki=P, two=2)
        # Issue order = consumption order of the in-order PE stream
        # [f0 f1 d0 f2 f3 d1 ...]: interleave the fp8 chunk loads k-wise with
        # the fp16 chunks instead of queueing all 4MB of fp16 first — else
        # the first DoubleRow matmul stalls the whole phase-1 chain ~6us.
        for k in range(KO):
            if b == 0:
                nc.sync.dma_start(W16_sb[k][:], W16_re[:, k, :])
            nc.sync.dma_start(qT16_sb[k][:], qT16_re[:, k, :])
            if k % 2 == 1:
                kk = k // 2
                if b == 0:
                    nc.sync.dma_start(W8_sb[kk][:], W8_re[:, kk, :, :])
                nc.sync.dma_start(qT8r_sb[kk][:], qT8r_re[:, kk, :, :])

        # ---- phase 1: keysT[h, lq] = (q @ W)^T, fp16 main + DR correction,
        # then split for MM2: k16 (fp16) + k8 (e5m2).
        k16_sb = keysT_pool.tile([P, KO, L], F16, name=f"k16_{b}", tag="k16")
        k8_sb = keysT_pool.tile([P, KO, L], E5, name=f"k8_{b}", tag="k8")
        for m in range(KO):
            ps_k = ps_big.tile([P, L], F32, name=f"ps_k_{b}_{m}", tag="ps_big")
            for n in range(NCH):
                for k in range(KO):
                    nc.tensor.matmul(
                        ps_k[:, ts(n, NF)],
                        W16_sb[k][:, ts(m, P)],
                        qT16_sb[k][:, ts(n, NF)],
                        start=(k == 0),
                        stop=False,
                    )
                    if k % 2 == 1:
                        kk = k // 2
                        nc.tensor.matmul(
                            ps_k[:, ts(n, NF)],
                            W8_sb[kk][:, :, ts(m, P)],
                            qT8r_sb[kk][:, :, ts(n, NF)],
                            start=False,
                            stop=(k == KO - 1),
                            perf_mode=DR,
                        )
            # split on two engines so the PSUM buffer frees in ~1.2us, not
            # 2.4us — the ps_big recycle gates the next m-group's matmuls
            nc.vector.tensor_copy(k16_sb[:, m, :], ps_k[:])
            nc.scalar.activation(k8_sb[:, m, :], ps_k[:], AF.Copy)

        # q natural (fp16, for MM3): issued after phase-1 compute so its DMA
        # queues drain behind the phase-1-critical loads.
        qn_sb = q_pool.tile([P, KO, H], F16, name=f"qn_sb_{b}", tag="qn_sb")
        qre = qn16[b].rearrange("(ko ki) h -> ki ko h", ki=P)
        for k in range(KO):
            nc.sync.dma_start(qn_sb[:, k, :], qre[:, k, :])

        # ---- phase 2/3: per lp tile, software-pipelined
        pT16_r = pT16[b].rearrange("(ko ki) l -> ki ko l", ki=P)
        pT8r_r = pT8r[b].rearrange("(kk two ki) l -> ki kk two l", ki=P, two=2)
        scores_ps = {}
        soft = {}

        def stage_scores(i, b=b, pT16_r=pT16_r, pT8r_r=pT8r_r,
                         k16_sb=k16_sb, k8_sb=k8_sb):
            p16_sb = pT_pool.tile([P, KO, P], F16, name=f"p16_sb_{b}_{i}",
                                  tag="p16_sb")
            p8r_sb = pT_pool.tile([P, KD, 2, P], E5, name=f"p8r_sb_{b}_{i}",
                                  tag="p8r_sb")
            nc.sync.dma_start(p16_sb[:], pT16_r[:, :, ts(i, P)])
            # one DMA per slot: a single 4-dim AP pair fails DMA balancing
            nc.sync.dma_start(p8r_sb[:, :, 0, :], pT8r_r[:, :, 0, ts(i, P)])
            nc.sync.dma_start(p8r_sb[:, :, 1, :], pT8r_r[:, :, 1, ts(i, P)])
            ps_s = ps_big.tile([P, L], F32, name=f"ps_s_{b}_{i}", tag="ps_big")
            for n in range(NCH):
                for k in range(KO):
                    nc.tensor.matmul(
                        ps_s[:, ts(n, NF)],
                        p16_sb[:, k, :],
                        k16_sb[:, k, ts(n, NF)],
                        start=(k == 0),
                        stop=False,
                    )
                    if k % 2 == 1:
                        kk = k // 2
                        nc.tensor.matmul(
                            ps_s[:, ts(n, NF)],
                            p8r_sb[:, kk, :, :],
                            k8_sb[:, 2 * kk : 2 * kk + 2, ts(n, NF)],
                            start=False,
                            stop=(k == KO - 1),
                            perf_mode=DR,
                        )
            scores_ps[i] = ps_s

        def stage_softmax_t(i, b=b):
            ps_s = scores_ps.pop(i)
            negmax = stat_pool.tile([P, 1], F32, name=f"negmax_{b}_{i}", tag="negmax")
            nc.vector.reduce_max(negmax[:], ps_s[:], axis=AX.X, negate=True)
            attn_sb = attn_pool.tile([P, L], F16, name=f"attn_{b}_{i}", tag="attn")
            rowsum = stat_pool.tile([P, 1], F32, name=f"rowsum_{b}_{i}", tag="rowsum")
            nc.scalar.activation(
                attn_sb[:],
                ps_s[:],
                AF.Exp,
                bias=negmax[:],
                accum_out=rowsum[:],
            )
            recip = stat_pool.tile([P, 1], F32, name=f"recip_{b}_{i}", tag="recip")
            nc.vector.reciprocal(recip[:], rowsum[:])

            attnT_sb = attnT_pool.tile([P, L], F16, name=f"attnT_{b}_{i}", tag="attnT")
            for g in range(L // NF):
                ps_tt = ps_t.tile([P, NF], F16, name=f"ps_tt_{b}_{i}_{g}", tag="ps_t")
                for j in range(NF // P):
                    c = g * (NF // P) + j
                    nc.tensor.transpose(
                        ps_tt[:, ts(j, P)], attn_sb[:, ts(c, P)], ident[:]
                    )
                nc.vector.tensor_copy(attnT_sb[:, ts(g, NF)], ps_tt[:])
            soft[i] = (attnT_sb, recip)

        def stage_mm3(i, b=b, qn_sb=qn_sb):
            attnT_sb, recip = soft.pop(i)
            out_sb = osb_pool.tile([P, H], F32, name=f"out_sb_{b}_{i}", tag="out_sb")
            ps_o = ps_big.tile([P, H], F32, name=f"ps_o_{b}_{i}", tag="ps_big")
            # relu+store per n-chunk so the drain of chunk 0 hides under the
            # matmuls of chunk 1 (shrinks the kernel tail).
            for n in range(NCH):
                for k in range(KO):
                    nc.tensor.matmul(
                        ps_o[:, ts(n, NF)],
                        attnT_sb[:, ts(k, P)],
                        qn_sb[:, k, ts(n, NF)],
                        start=(k == 0),
                        stop=(k == KO - 1),
                    )
                nc.scalar.activation(
                    out_sb[:, ts(n, NF)], ps_o[:, ts(n, NF)], AF.Relu, scale=recip[:]
                )
                nc.sync.dma_start(out[b, ts(i, P), ts(n, NF)], out_sb[:, ts(n, NF)])

        stage_scores(0)
        stage_scores(1)
        for i in range(NT):
            stage_softmax_t(i)
            if i + 2 < NT:
                stage_scores(i + 2)
            stage_mm3(i)


_IN_NAMES = ["pT16", "pT8r", "qT16", "qT8r", "qn16", "W16", "W8"]

_CACHED = None


def _get_program():
    global _CACHED
    if _CACHED is not None:
        return _CACHED
    nc = bacc.Bacc(
        "TRN2",
        target_bir_lowering=False,
        debug=False,
        num_devices=NCORES,
    )
    specs = {
        "pT16": ([BPC, H, L], F16),
        "pT8r": ([BPC, H, L], E5),
        "qT16": ([BPC, H, L], F16),
        "qT8r": ([BPC, H, L], E5),
        "qn16": ([BPC, L, H], F16),
        "W16": ([H, H], F16),
        "W8": ([H, H], E5),
    }
    handles = [
        nc.dram_tensor(name, *specs[name], kind="ExternalInput") for name in _IN_NAMES
    ]
    out_h = nc.dram_tensor("out", [BPC, L, H], F32, kind="ExternalOutput")
    with tile.TileContext(nc) as tc:
        with ExitStack() as ctx:
            _build_body(ctx, tc, [h.ap() for h in handles], out_h.ap())
    nc.compile()
    _CACHED = nc
    return nc


def kernel(p, q, W_key, b_key):
    # b_key is mathematically irrelevant: softmax over lq is invariant to the
    # per-lp constant p@b^T it adds to scores, and keys are not used elsewhere.
    del b_key
    E5np = ml_dtypes.float8_e5m2
    p = np.ascontiguousarray(np.asarray(p, dtype=np.float32))
    q = np.ascontiguousarray(np.asarray(q, dtype=np.float32))
    W = np.ascontiguousarray(np.asarray(W_key, dtype=np.float32))
    pT = np.ascontiguousarray(p.transpose(0, 2, 1))
    qT = np.ascontiguousarray(q.transpose(0, 2, 1))

    pT16 = pT.astype(np.float16)
    pT8r = (pT - pT16.astype(np.float32)).astype(E5np)
    qT16 = qT.astype(np.float16)
    qT8r = (qT - qT16.astype(np.float32)).astype(E5np)
    qn16 = q.astype(np.float16)
    W16 = W.astype(np.float16)
    W8 = W16.astype(np.float32).astype(E5np)

    full = {
        "pT16": pT16, "pT8r": pT8r,
        "qT16": qT16, "qT8r": qT8r,
        "qn16": qn16,
    }

    in_maps = []
    for c in range(NCORES):
        sl = slice(c * BPC, (c + 1) * BPC)
        m = {k: np.ascontiguousarray(v[sl]) for k, v in full.items()}
        m["W16"] = W16
        m["W8"] = W8
        in_maps.append(m)

    nc = _get_program()
    trace = bool(int(os.environ.get("MATCHNET_TRACE", "0")))
    res = run_bass_kernel_spmd(nc, in_maps, list(range(NCORES)), trace=trace)
    if trace:
        kernel.last_exec_time_ns = res.exec_time_ns
        kernel.last_results = res
    out = np.concatenate([res.results[c]["out"] for c in range(NCORES)], axis=0)
    return out


kernel.last_exec_time_ns = None
kernel.last_results = None


# revision 8
# speedup vs baseline: 1.0128x; 1.0128x over previous
"""MatchNet kernel for 8 Trainium2 NeuronCores.

Math (per batch b):
    keys   = q[b] @ W + bias
    scores = p[b] @ keys^T
    attn   = softmax(scores, axis=-1)
    out[b] = relu(attn @ q[b])

The Dense bias is dropped: softmax over lq is invariant to the per-lp
constant p@b^T it adds to scores, and keys are not used elsewhere.

Sharding: data-parallel over B=16 across 8 cores (2 batches per core).
W is broadcast. p and q are transposed on the host so every on-chip matmul
has its contraction dim on SBUF partitions.

Precision: the softmax is extremely sharp (scores std ~32), so plain
bf16/fp16 matmuls in the score path are not accurate enough. Each score-path
matmul runs as:
  - 1 fp16 "main" pass: x16 @ y16 at full PE rate (8 k-chunk matmuls), and
  - 1 single-term fp8-e5m2 correction at 2x PE rate via DoubleRow, with the
    pair slots carrying two CONSECUTIVE contraction chunks (4 matmuls):
      MM1 += qr8 @ W8        (q residual vs fp16, e5m2)
      MM2 += pr8 @ k8        (p residual vs fp16, e5m2)
    DoubleRow computes sum_i lhsT[:,i,:].T @ rhs[:,i,:].
The other two correction terms (W residual, keys residual) are dropped:
numpy sim of this exact scheme on the real inputs: rel_err 1.0e-2
(gate 2e-2; HW has measured slightly BETTER than sim on two prior configs).
    MM1: keysT[h, lq] = sum_hk W[hk, h] * qT[hk, lq]
         keysT split on-chip: k16 (fp16) + k8 (e5m2) via DVE copies
    MM2: scores[lp, lq] = sum_h pT[h, lp] * keysT[h, lq]
    softmax over free dim; exp via ACT (bias=-rowmax, accum rowsum),
    exp output stored fp16
    T:   attnT[lq, lp] via PE transpose (fp16)
    MM3: out[lp, h] = sum_lq attnT[lq, lp] * q[lq, h]  single fp16 pass
    relu(out * (1/rowsum)) via ACT with per-partition scale
(DMA xbar transpose for attnT was tried instead of PE transposes and was
~110us SLOWER end-to-end — keep the PE-transpose path.)
"""

import os
from contextlib import ExitStack

import ml_dtypes
import numpy as np

import concourse.bass as bass
import concourse.mybir as mybir
import concourse.tile as tile
from concourse import bacc
from concourse.bass import ts
from concourse.bass_utils import run_bass_kernel_spmd
from concourse.masks import make_identity

B, L, H = 16, 1024, 1024
NCORES = 8
BPC = B // NCORES  # batches per core
P = 128
KO = H // P        # 8 contraction chunks
KD = KO // 2       # 4 DoubleRow chunk-pairs
NT = L // P        # 8 lp tiles per batch
NF = 512           # matmul moving free dim
NCH = L // NF      # 2 free chunks
F32 = mybir.dt.float32
F16 = mybir.dt.float16
E5 = mybir.dt.float8e5
AF = mybir.ActivationFunctionType
AX = mybir.AxisListType
DR = mybir.MatmulPerfMode.DoubleRow


def _build_body(ctx, tc, ins, out):
    nc = tc.nc
    pT16, pT8r, qT16, qT8r, qn16, W16, W8 = ins

    # PE warmup: the first ~15us are DMA-bound (bootstrap + first loads) and
    # the PE would sit idle, entering the kernel HAM-throttled at 1.2 GHz.
    # Zero matmuls during that window cost nothing and flip the clock gate
    # to 2.4 GHz before the real matmuls start.
    with (
        tc.tile_pool(name="warm", bufs=1) as warm_pool,
        tc.tile_pool(name="warmps", bufs=1, space=bass.MemorySpace.PSUM) as wps_pool,
    ):
        wsb = warm_pool.tile([P, P], F16)
        nc.gpsimd.memset(wsb[:], 0.0)
        wps = wps_pool.tile([P, P], F32)
        for _ in range(60):
            nc.tensor.matmul(wps[:], wsb[:], wsb[:], start=True, stop=True)

    const = ctx.enter_context(tc.tile_pool(name="const", bufs=1))
    # W tiles, one per k-chunk (chunk-granular deps let the first matmul
    # start as soon as chunk 0 lands instead of after the full load).
    W16_sb = [const.tile([P, H], F16, name=f"W16_sb_{k}") for k in range(KO)]
    # W8 pair tiles: slot s holds W8 rows [256*kk + 128*s, +128)
    W8_sb = [const.tile([P, 2, H], E5, name=f"W8_sb_{kk}") for kk in range(KD)]
    ident = const.tile([P, P], F16)
    make_identity(nc, ident[:])

    qT_pool = ctx.enter_context(tc.tile_pool(name="qTp", bufs=1))
    q_pool = ctx.enter_context(tc.tile_pool(name="qp", bufs=1))
    keysT_pool = ctx.enter_context(tc.tile_pool(name="keysTp", bufs=1))
    pT_pool = ctx.enter_context(tc.tile_pool(name="pTp", bufs=3))
    attn_pool = ctx.enter_context(tc.tile_pool(name="attnp", bufs=2))
    attnT_pool = ctx.enter_context(tc.tile_pool(name="attnTp", bufs=2))
    osb_pool = ctx.enter_context(tc.tile_pool(name="osbp", bufs=2))
    stat_pool = ctx.enter_context(tc.tile_pool(name="statp", bufs=8))
    ps_big = ctx.enter_context(
        tc.tile_pool(name="psbig", bufs=3, space=bass.MemorySpace.PSUM)
    )
    ps_t = ctx.enter_context(
        tc.tile_pool(name="pst", bufs=2, space=bass.MemorySpace.PSUM)
    )

    W16_re = W16.rearrange("(ko ki) h -> ki ko h", ki=P)
    W8_re = W8.rearrange("(kk two ki) h -> ki kk two h", ki=P, two=2)

    for b in range(BPC):
        # qT tiles first (MM1 needs them); fp16 mains before fp8 residuals so
        # the first fp16 matmul of each k-chunk can start while residuals load.
        qT16_sb = [
            qT_pool.tile([P, L], F16, name=f"qT16_sb_{b}_{k}", tag=f"qT16_sb{k}")
            for k in range(KO)
        ]
        qT8r_sb = [
            qT_pool.tile([P, 2, L], E5, name=f"qT8r_sb_{b}_{kk}", tag=f"qT8r_sb{kk}")
            for kk in range(KD)
        ]
        qT16_re = qT16[b].rearrange("(ko ki) l -> ki ko l", ki=P)
        qT8r_re = qT8r[b].rearrange("(kk two ki) l -> ki kk two l", ki=P, two=2)
        # Issue order = consumption order of the in-order PE stream
        # [f0 f1 d0 f2 f3 d1 ...]: interleave the fp8 chunk loads k-wise with
        # the fp16 chunks instead of queueing all 4MB of fp16 first — else
        # the first DoubleRow matmul stalls the whole phase-1 chain ~6us.
        for k in range(KO):
            if b == 0:
                nc.sync.dma_start(W16_sb[k][:], W16_re[:, k, :])
            nc.sync.dma_start(qT16_sb[k][:], qT16_re[:, k, :])
            if k % 2 == 1:
                kk = k // 2
                if b == 0:
                    nc.sync.dma_start(W8_sb[kk][:], W8_re[:, kk, :, :])
                nc.sync.dma_start(qT8r_sb[kk][:], qT8r_re[:, kk, :, :])

        # ---- phase 1: keysT[h, lq] = (q @ W)^T, fp16 main + DR correction,
        # then split for MM2: k16 (fp16) + k8 (e5m2).
        k16_sb = keysT_pool.tile([P, KO, L], F16, name=f"k16_{b}", tag="k16")
        k8_sb = keysT_pool.tile([P, KO, L], E5, name=f"k8_{b}", tag="k8")
        # Each m-group is split into half-contraction sub-groups A (chunks
        # 0-3) and B (chunks 4-7), issued A0 A1 A2 B0 A3 B1 ... so the PE has
        # ~21us of A-work gated only on the first half of the 6MB phase-1
        # stream (the kernel start is DMA-bandwidth-bound). Max 3 PSUM tiles
        # live (m..m+2) matches ps_big bufs=3.
        ps_ks = {}

        def phase1_half(m, half):
            if half == 0:
                ps_ks[m] = ps_big.tile([P, L], F32, name=f"ps_k_{b}_{m}",
                                       tag="ps_big")
            ps_k = ps_ks[m]
            for n in range(NCH):
                for k in range(4 * half, 4 * half + 4):
                    nc.tensor.matmul(
                        ps_k[:, ts(n, NF)],
                        W16_sb[k][:, ts(m, P)],
                        qT16_sb[k][:, ts(n, NF)],
                        start=(k == 0),
                        stop=False,
                    )
                    if k % 2 == 1:
                        kk = k // 2
                        nc.tensor.matmul(
                            ps_k[:, ts(n, NF)],
                            W8_sb[kk][:, :, ts(m, P)],
                            qT8r_sb[kk][:, :, ts(n, NF)],
                            start=False,
                            stop=(k == KO - 1),
                            perf_mode=DR,
                        )
            if half == 1:
                ps_k = ps_ks.pop(m)
                # split on two engines so the PSUM buffer frees in ~1.2us,
                # not 2.4us — ps_big recycle gates later m-groups' matmuls
                nc.vector.tensor_copy(k16_sb[:, m, :], ps_k[:])
                nc.scalar.activation(k8_sb[:, m, :], ps_k[:], AF.Copy)

        for step in range(KO + 3):
            if step >= 3:
                phase1_half(step - 3, 1)
            if step < KO:
                phase1_half(step, 0)

        # q natural (fp16, for MM3): issued after phase-1 compute so its DMA
        # queues drain behind the phase-1-critical loads.
        qn_sb = q_pool.tile([P, KO, H], F16, name=f"qn_sb_{b}", tag="qn_sb")
        qre = qn16[b].rearrange("(ko ki) h -> ki ko h", ki=P)
        for k in range(KO):
            nc.sync.dma_start(qn_sb[:, k, :], qre[:, k, :])

        # ---- phase 2/3: per lp tile, software-pipelined
        pT16_r = pT16[b].rearrange("(ko ki) l -> ki ko l", ki=P)
        pT8r_r = pT8r[b].rearrange("(kk two ki) l -> ki kk two l", ki=P, two=2)
        scores_ps = {}
        soft = {}

        def stage_scores(i, b=b, pT16_r=pT16_r, pT8r_r=pT8r_r,
                         k16_sb=k16_sb, k8_sb=k8_sb):
            p16_sb = pT_pool.tile([P, KO, P], F16, name=f"p16_sb_{b}_{i}",
                                  tag="p16_sb")
            p8r_sb = pT_pool.tile([P, KD, 2, P], E5, name=f"p8r_sb_{b}_{i}",
                                  tag="p8r_sb")
            nc.sync.dma_start(p16_sb[:], pT16_r[:, :, ts(i, P)])
            # one DMA per slot: a single 4-dim AP pair fails DMA balancing
            nc.sync.dma_start(p8r_sb[:, :, 0, :], pT8r_r[:, :, 0, ts(i, P)])
            nc.sync.dma_start(p8r_sb[:, :, 1, :], pT8r_r[:, :, 1, ts(i, P)])
            ps_s = ps_big.tile([P, L], F32, name=f"ps_s_{b}_{i}", tag="ps_big")
            for n in range(NCH):
                for k in range(KO):
                    nc.tensor.matmul(
                        ps_s[:, ts(n, NF)],
                        p16_sb[:, k, :],
                        k16_sb[:, k, ts(n, NF)],
                        start=(k == 0),
                        stop=False,
                    )
                    if k % 2 == 1:
                        kk = k // 2
                        nc.tensor.matmul(
                            ps_s[:, ts(n, NF)],
                            p8r_sb[:, kk, :, :],
                            k8_sb[:, 2 * kk : 2 * kk + 2, ts(n, NF)],
                            start=False,
                            stop=(k == KO - 1),
                            perf_mode=DR,
                        )
            scores_ps[i] = ps_s

        def stage_softmax_t(i, b=b):
            ps_s = scores_ps.pop(i)
            negmax = stat_pool.tile([P, 1], F32, name=f"negmax_{b}_{i}", tag="negmax")
            nc.vector.reduce_max(negmax[:], ps_s[:], axis=AX.X, negate=True)
            attn_sb = attn_pool.tile([P, L], F16, name=f"attn_{b}_{i}", tag="attn")
            rowsum = stat_pool.tile([P, 1], F32, name=f"rowsum_{b}_{i}", tag="rowsum")
            nc.scalar.activation(
                attn_sb[:],
                ps_s[:],
                AF.Exp,
                bias=negmax[:],
                accum_out=rowsum[:],
            )
            recip = stat_pool.tile([P, 1], F32, name=f"recip_{b}_{i}", tag="recip")
            nc.vector.reciprocal(recip[:], rowsum[:])

            attnT_sb = attnT_pool.tile([P, L], F16, name=f"attnT_{b}_{i}", tag="attnT")
            for g in range(L // NF):
                ps_tt = ps_t.tile([P, NF], F16, name=f"ps_tt_{b}_{i}_{g}", tag="ps_t")
                for j in range(NF // P):
                    c = g * (NF // P) + j
                    nc.tensor.transpose(
                        ps_tt[:, ts(j, P)], attn_sb[:, ts(c, P)], ident[:]
                    )
                nc.vector.tensor_copy(attnT_sb[:, ts(g, NF)], ps_tt[:])
            soft[i] = (attnT_sb, recip)

        def stage_mm3(i, b=b, qn_sb=qn_sb):
            attnT_sb, recip = soft.pop(i)
            out_sb = osb_pool.tile([P, H], F32, name=f"out_sb_{b}_{i}", tag="out_sb")
            ps_o = ps_big.tile([P, H], F32, name=f"ps_o_{b}_{i}", tag="ps_big")
            # relu+store per n-chunk so the drain of chunk 0 hides under the
            # matmuls of chunk 1 (shrinks the kernel tail).
            for n in range(NCH):
                for k in range(KO):
                    nc.tensor.matmul(
                        ps_o[:, ts(n, NF)],
                        attnT_sb[:, ts(k, P)],
                        qn_sb[:, k, ts(n, NF)],
                        start=(k == 0),
                        stop=(k == KO - 1),
                    )
                nc.scalar.activation(
                    out_sb[:, ts(n, NF)], ps_o[:, ts(n, NF)], AF.Relu, scale=recip[:]
                )
                nc.sync.dma_start(out[b, ts(i, P), ts(n, NF)], out_sb[:, ts(n, NF)])

        stage_scores(0)
        stage_scores(1)
        for i in range(NT):
            stage_softmax_t(i)
            if i + 2 < NT:
                stage_scores(i + 2)
            stage_mm3(i)


_IN_NAMES = ["pT16", "pT8r", "qT16", "qT8r", "qn16", "W16", "W8"]

_CACHED = None


def _get_program():
    global _CACHED
    if _CACHED is not None:
        return _CACHED
    nc = bacc.Bacc(
        "TRN2",
        target_bir_lowering=False,
        debug=False,
        num_devices=NCORES,
    )
    specs = {
        "pT16": ([BPC, H, L], F16),
        "pT8r": ([BPC, H, L], E5),
        "qT16": ([BPC, H, L], F16),
        "qT8r": ([BPC, H, L], E5),
        "qn16": ([BPC, L, H], F16),
        "W16": ([H, H], F16),
        "W8": ([H, H], E5),
    }
    handles = [
        nc.dram_tensor(name, *specs[name], kind="ExternalInput") for name in _IN_NAMES
    ]
    out_h = nc.dram_tensor("out", [BPC, L, H], F32, kind="ExternalOutput")
    with tile.TileContext(nc) as tc:
        with ExitStack() as ctx:
            _build_body(ctx, tc, [h.ap() for h in handles], out_h.ap())
    nc.compile()
    _CACHED = nc
    return nc


def kernel(p, q, W_key, b_key):
    # b_key is mathematically irrelevant: softmax over lq is invariant to the
    # per-lp constant p@b^T it adds to scores, and keys are not used elsewhere.
    del b_key
    E5np = ml_dtypes.float8_e5m2
    p = np.ascontiguousarray(np.asarray(p, dtype=np.float32))
    q = np.ascontiguousarray(np.asarray(q, dtype=np.float32))
    W = np.ascontiguousarray(np.asarray(W_key, dtype=np.float32))
    pT = np.ascontiguousarray(p.transpose(0, 2, 1))
    qT = np.ascontiguousarray(q.transpose(0, 2, 1))

    pT16 = pT.astype(np.float16)
    pT8r = (pT - pT16.astype(np.float32)).astype(E5np)
    qT16 = qT.astype(np.float16)
    qT8r = (qT - qT16.astype(np.float32)).astype(E5np)
    qn16 = q.astype(np.float16)
    W16 = W.astype(np.float16)
    W8 = W16.astype(np.float32).astype(E5np)

    full = {
        "pT16": pT16, "pT8r": pT8r,
        "qT16": qT16, "qT8r": qT8r,
        "qn16": qn16,
    }

    in_maps = []
    for c in range(NCORES):
        sl = slice(c * BPC, (c + 1) * BPC)
        m = {k: np.ascontiguousarray(v[sl]) for k, v in full.items()}
        m["W16"] = W16
        m["W8"] = W8
        in_maps.append(m)

    nc = _get_program()
    trace = bool(int(os.environ.get("MATCHNET_TRACE", "0")))
    res = run_bass_kernel_spmd(nc, in_maps, list(range(NCORES)), trace=trace)
    if trace:
        kernel.last_exec_time_ns = res.exec_time_ns
        kernel.last_results = res
    out = np.concatenate([res.results[c]["out"] for c in range(NCORES)], axis=0)
    return out


kernel.last_exec_time_ns = None
kernel.last_results = None
